# revision 12
# baseline (speedup 1.0000x reference)
"""Cosine-similarity self-attention (Cos_Attn) on 8 Trainium2 NeuronCores.

Reference math (x: [C=512, W=64, H=64] fp32, N = W*H = 4096):
    q = x.reshape(C, N).T                  # [N, C]
    energy = q @ q.T                       # [N, N]
    cos    = energy / (|q_i| |q_j|)
    out    = softmax(cos, axis=-1)[None]   # [1, N, N]

Sharding: N query rows split across 8 cores (512 rows each). One shared
program; per-core asymmetry is handled by ROTATING the input data so that
device-block 0 is always the core's own query block. Host un-rotates the
output columns.

v5 design (per core). Measured engine facts this design is built on:
DVE tensor ops run 2x only when every tensor operand is 2-byte (fp8
anywhere -> 1x, measured 2.1us/block); fp8 DoubleRow matmul works and
halves PE cycles/row; 2048-wide exps cost 1.92us; each Sqrt<->Exp ACT
table switch costs 1.28us (compiler picks tables greedily).
  - input x bf16, block-major [NB=8, P=128, KO=4, CB=512].
  - per block: DVE squares (bf16 2x) -> PE bf16 ones-matmul column sums
    (PSUM f32) -> DVE reciprocal_approx_fast -> ACT Sqrt(scale=64) on
    block PAIRS -> rn = 8/|q| bf16 -> DVE normalize xn = x*rn (bf16 2x).
  - GpSimd (otherwise idle) casts xn -> fp8 per block, off the critical
    path; the 8x key scale uses e4m3 range, undone by the 1/64 exp scale.
  - energy: PE fp8 DoubleRow on the casted keys, 4-bank PSUM groups
    [P, 4, CB]; ns shares the same PSUM pool tag so 2x4 banks
    double-buffer cleanly (PSUM is exactly 8 banks).
  - softmax: ACT Exp(scale=1/64) from PSUM, 2048 wide, bf16 out,
    accum_out row sums; max-subtraction skipped (cos bounded).
  - ACT order: ALL sqrts strictly before ALL exps (2 table loads); exps
    g-major so the fp8 cast pace (2.8us/block on GpSimd) stays ahead of
    the exp chain; row tails right after each m's second exp.
  - row scale 1/rowsum: ACT Copy rr_row (free-affine per-partition
    scale; Copy lives in every table set) + DVE bf16 multiply; output
    bf16 [MT, P, N], host upcasts.
"""

import numpy as np

_NCORES = 8
_P = 128

# set by the test harness only; the grading path keeps these defaults
TRACE = False
TRACE_CORES = None
LAST_RESULT = None

_built = None  # (nc, C, N)

GB = 4            # blocks per energy group (PSUM banks per tile)
WARMUP_MM = 6     # junk matmuls to ramp the PE p-state during DMA wait
PT_BUFS = 2
FP8_GS = (1,)     # energy groups computed in fp8 DoubleRow (others bf16)
OUT_DMA = "sync"  # engine issuing output DMAs


def _build(C, N):
    from contextlib import ExitStack

    import concourse.tile as tile
    from concourse import bacc, mybir

    f32 = mybir.dt.float32
    bf16 = mybir.dt.bfloat16
    fp8 = mybir.dt.float8e4
    AF = mybir.ActivationFunctionType
    AX = mybir.AxisListType
    OP = mybir.AluOpType
    DR = mybir.MatmulPerfMode.DoubleRow

    P = _P
    KO = C // P              # contraction subtiles (4)
    KP = KO // 2             # DoubleRow k-pairs (2)
    CB = 512                 # column block = one PSUM bank of f32
    NB = N // CB             # 8 column blocks
    MT = (N // _NCORES) // P # 4 query row tiles per core
    NG = NB // GB            # 2 energy groups per row tile

    nc = bacc.Bacc("TRN2", target_bir_lowering=False, debug=False)
    x_d = nc.dram_tensor("x", [NB, P, KO, CB], bf16, kind="ExternalInput")
    out_d = nc.dram_tensor("out", [MT, P, N], bf16, kind="ExternalOutput")

    with tile.TileContext(nc) as tc, ExitStack() as ctx:
        persist = ctx.enter_context(tc.tile_pool(name="persist", bufs=1))
        temps = ctx.enter_context(tc.tile_pool(name="temps", bufs=3))
        psum = ctx.enter_context(tc.tile_pool(name="psum", bufs=2, space="PSUM"))

        xnb = persist.tile([P, KO, N], bf16)     # normalized keys, bf16
        xn8 = persist.tile([P, KO, N], fp8)      # fp8 cast (GpSimd)
        e = persist.tile([P, MT, N], bf16)       # exp(cos); scaled in place
        rn = persist.tile([P, N], bf16)          # 8/|q_j| replicated on parts
        sums = persist.tile([P, MT, NG], f32)    # per-(m, g) exp row sums
        rs = persist.tile([P, MT], f32)
        rr = persist.tile([P, MT], f32)
        ones = persist.tile([P, P], bf16)
        ones_row = persist.tile([P, CB], f32)
        nc.vector.memset(ones[:], 1.0)
        nc.vector.memset(ones_row[:], 1.0)

        xr_tiles = {}

        def dma_in(b):
            xr = temps.tile([P, KO, CB], bf16, tag="xr", name="xr", bufs=8)
            nc.sync.dma_start(xr[:], x_d.ap()[b])
            xr_tiles[b] = xr

        def pt_alloc(name):
            return psum.tile([P, GB, CB], f32, tag="pt", name=name,
                             bufs=PT_BUFS)

        def warmup_pe():
            junk = pt_alloc("junk")
            for i in range(WARMUP_MM):
                nc.tensor.matmul(junk[:, 0, 0:P], lhsT=ones[:], rhs=ones[:],
                                 start=(i == 0), stop=(i == WARMUP_MM - 1))

        def norm_pre(b, r1pair):
            """squares -> colsum matmul -> approx reciprocal (no ACT)."""
            xsq = temps.tile([P, KO, CB], bf16, tag="xsq", name="xsq", bufs=3)
            nc.vector.tensor_mul(xsq[:], xr_tiles[b][:], xr_tiles[b][:])
            ns = pt_alloc("ns")
            for k in range(KO):
                nc.tensor.matmul(
                    ns[:, 0, :], lhsT=ones[:], rhs=xsq[:, k, :],
                    start=(k == 0), stop=(k == KO - 1),
                )
            nc.vector.reciprocal_approx_fast(r1pair[:, b % 2, :], ns[:, 0, :])

        def pair_sqrt(bp):
            """rn[pair] = sqrt(64 * r1pair) = 8/|q| for blocks 2bp, 2bp+1."""
            cs = slice(2 * bp * CB, (2 * bp + 2) * CB)
            nc.scalar.activation(
                rn[:, cs], r1pairs[bp][:].rearrange("p a b -> p (a b)"),
                AF.Sqrt, scale=64.0)

        def block_normalize(b):
            cs = slice(b * CB, (b + 1) * CB)
            rn_b = rn[:, None, cs].to_broadcast([P, KO, CB])
            nc.vector.tensor_mul(xnb[:, :, cs], xr_tiles.pop(b)[:], rn_b)

        def block_cast(b):
            # block 0 is the query slice: always needed in fp8 as lhsT
            if b != 0 and (b // GB) not in FP8_GS:
                return  # this block's energy group runs bf16 off xnb
            cs = slice(b * CB, (b + 1) * CB)
            nc.gpsimd.tensor_copy(xn8[:, :, cs], xnb[:, :, cs])

        def energy_group(m, g):
            ms = slice(m * P, (m + 1) * P)
            pt = pt_alloc("pt")
            if g in FP8_GS:
                for kp in range(KP):
                    ks = slice(2 * kp, 2 * kp + 2)
                    for j in range(GB):
                        b = g * GB + j
                        cs = slice(b * CB, (b + 1) * CB)
                        nc.tensor.matmul(
                            pt[:, j, :],
                            lhsT=xn8[:, ks, ms],  # queries = block-0 cols
                            rhs=xn8[:, ks, cs],
                            start=(kp == 0), stop=(kp == KP - 1),
                            perf_mode=DR,
                        )
            else:
                for k in range(KO):
                    for j in range(GB):
                        b = g * GB + j
                        cs = slice(b * CB, (b + 1) * CB)
                        nc.tensor.matmul(
                            pt[:, j, :],
                            lhsT=xnb[:, k, ms],
                            rhs=xnb[:, k, cs],
                            start=(k == 0), stop=(k == KO - 1),
                        )
            return pt

        def exp_group(m, g, pt):
            gs = slice(g * GB * CB, (g + 1) * GB * CB)
            nc.scalar.activation(
                e[:, m, gs], pt[:].rearrange("p a b -> p (a b)"), AF.Exp,
                scale=1.0 / 64.0, accum_out=sums[:, m, g:g + 1],
            )

        def tail(m):
            """row scale + output DMA for row tile m."""
            nc.vector.tensor_reduce(
                rs[:, m:m + 1], sums[:, m, :], axis=AX.X, op=OP.add)
            nc.vector.reciprocal(rr[:, m:m + 1], rs[:, m:m + 1])
            rr_row = temps.tile([P, CB], bf16, tag="rr_row", name="rr_row",
                                bufs=2)
            nc.scalar.activation(rr_row[:], ones_row[:], AF.Copy,
                                 scale=rr[:, m:m + 1])
            HC = N // 2
            for h in range(2):
                hs = slice(h * HC, (h + 1) * HC)
                ev = e[:, m, hs].rearrange("p (a b) -> p a b", b=CB)
                rr_b = rr_row[:, None, :].to_broadcast([P, HC // CB, CB])
                nc.vector.tensor_mul(ev, ev, rr_b)
                eng = nc.sync if OUT_DMA == "sync" else nc.gpsimd
                eng.dma_start(out_d.ap()[m][:, hs], e[:, m, hs])

        # ---- emission; per-engine queue order is what matters ----
        for b in range(NB):
            dma_in(b)
        warmup_pe()

        r1pairs = {}
        for bp in range(NB // 2):
            r1pairs[bp] = temps.tile([P, 2, CB], f32, tag="r1", name="r1",
                                     bufs=4)

        # phase A: per-block norm chains; sqrt on pairs; normalize right
        # after its pair's sqrt; GpSimd casts to fp8 as normalize lands
        for b in range(NB):
            norm_pre(b, r1pairs[b // 2])
            if b % 2 == 1:
                pair_sqrt(b // 2)
                block_normalize(b - 1)
                block_cast(b - 1)
                block_normalize(b)
                block_cast(b)

        # phase B: energy + exp, g-major (the fp8 cast pace stays ahead of
        # the exp chain); tails as soon as a row tile's g1 exp is done
        for g in range(NG):
            for m in range(MT):
                pt = energy_group(m, g)
                exp_group(m, g, pt)
                if g == NG - 1:
                    tail(m)

    nc.compile()
    return nc


def kernel(**inputs) -> np.ndarray:
    global _built, LAST_RESULT
    import ml_dtypes

    x = np.asarray(inputs["x"], dtype=np.float32)
    C, W, H = x.shape
    N = W * H
    P = _P
    KO = C // P
    CB = 512
    NB = N // CB
    MT = (N // _NCORES) // P

    if _built is None or _built[1:] != (C, N):
        _built = (_build(C, N), C, N)
    nc = _built[0]

    from concourse import bass_utils

    # block-major bf16 layout: xin[b, p, ko, c] = x[ko*128+p, b*512+c]
    x2 = x.reshape(KO, P, NB, CB)
    xin = np.ascontiguousarray(
        x2.transpose(2, 1, 0, 3)).astype(ml_dtypes.bfloat16)

    in_maps = [
        {"x": np.ascontiguousarray(np.roll(xin, -c, axis=0))}
        for c in range(_NCORES)
    ]
    kwargs = {}
    if TRACE:
        kwargs["trace"] = True
        if TRACE_CORES is not None:
            kwargs["trace_cores"] = list(TRACE_CORES)
    res = bass_utils.run_bass_kernel_spmd(
        nc, in_maps, core_ids=list(range(_NCORES)), **kwargs
    )
    LAST_RESULT = res
    out = np.empty((N, N), dtype=np.float32)
    for c in range(_NCORES):
        oc = np.asarray(res.results[c]["out"]).astype(np.float32)
        oc = oc.reshape(MT * P, N)          # rows of this core, rotated cols
        out[c * MT * P:(c + 1) * MT * P] = np.roll(oc, c * CB, axis=1)
    return out.reshape(1, N, N)
